# revision 4
# baseline (speedup 1.0000x reference)
"""AtlasNet sphere EdgeConv generator — Trainium2 Bass kernel (8-core SPMD).

Math (per batch element b, all on one NeuronCore; B=8 across 8 cores):

  EdgeConv layer with conv-before-activation and mean over K neighbors:
     out[:, n] = mean_k lrelu(W @ g[:, n, k] + b),  g[:, n, k] = x[:, idx[n,k]]
  Since conv1x1 is per-column and lrelu commutes with gather,
     out = lrelu(W @ x + b) @ S
  where S[m, n] = #{k : idx[n,k] = m} / K  — a dense (N, N) averaging matrix
  (entries k/8 are exact in bf16), applied as a PE matmul.

  Layer 1: f1 = [center; nbr-center; z] with W1 = [W1c | W1r | W1z]:
     h[:, n, k] = W6 @ [xyz[:, n]; xyz[:, idx[n,k]]] + (W1z @ z + b1)
  with W6 = [W1c - W1r | W1r]  (rank-6 contraction), and
     out1 = sum_k lrelu(h/8)   (lrelu is positively homogeneous -> fold 1/8)

  Layer 4 batch-norm folds into a per-channel scale/bias on the activation.
"""

import sys

sys.path.insert(0, "/opt/trn_rl_repo")
sys.path.insert(0, "/root/problem")

import numpy as np
import ml_dtypes

import tile_patch

tile_patch.install()

import concourse.bass as bass
import concourse.mybir as mybir
import concourse.tile as tile
from concourse.bass_utils import run_bass_kernel_spmd

F32 = mybir.dt.float32
BF16 = mybir.dt.bfloat16
AF = mybir.ActivationFunctionType
ALU = mybir.AluOpType

N = 2048          # points
K = 8             # knn
C = 512           # channels
NK = N * K        # pair count
P = 128           # partitions
CT = C // P       # channel tiles (4)
NT = N // P       # point tiles (16)
BN_EPS = 1e-5

_CACHED = {}


def _build_program():
    nc = bass.Bass()

    # ---- dram parameters (per-core; replicated except z / outputs) ----
    d_S = nc.declare_dram_parameter("S", [N, N], BF16, isOutput=False)
    d_Q = nc.declare_dram_parameter("Q", [8, NK], BF16, isOutput=False)
    d_W6T = nc.declare_dram_parameter("W6T", [8, C], BF16, isOutput=False)
    d_W1zT = nc.declare_dram_parameter("W1zT", [C, C], BF16, isOutput=False)
    d_W2T = nc.declare_dram_parameter("W2T", [C, C], BF16, isOutput=False)
    d_W3T = nc.declare_dram_parameter("W3T", [C, C], BF16, isOutput=False)
    d_W4T = nc.declare_dram_parameter("W4T", [C, C], BF16, isOutput=False)
    d_Wx1T = nc.declare_dram_parameter("Wx1T", [C, 64], BF16, isOutput=False)
    d_Wx2T = nc.declare_dram_parameter("Wx2T", [64, 3], F32, isOutput=False)
    d_z = nc.declare_dram_parameter("z", [C], BF16, isOutput=False)
    d_b1_8 = nc.declare_dram_parameter("b1_8", [C], F32, isOutput=False)
    d_b2 = nc.declare_dram_parameter("b2", [C], F32, isOutput=False)
    d_b3 = nc.declare_dram_parameter("b3", [C], F32, isOutput=False)
    d_s4 = nc.declare_dram_parameter("s4", [C], F32, isOutput=False)
    d_b4p = nc.declare_dram_parameter("b4p", [C], F32, isOutput=False)
    d_bx1 = nc.declare_dram_parameter("bx1", [64], F32, isOutput=False)
    d_bx2 = nc.declare_dram_parameter("bx2", [3], F32, isOutput=False)

    d_feat = nc.declare_dram_parameter("feature", [C, N], F32, isOutput=True)
    d_pcs = nc.declare_dram_parameter("pcs", [3, N], F32, isOutput=True)

    ctile = lambda d: d.rearrange("(t p) -> p t", p=P)  # (C,) -> (128, CT)
    cmat = lambda d: d.rearrange("(t p) m -> p t m", p=P)  # (C, M) -> (128, CT, M)

    with tile.TileContext(nc) as tc:
        with (
            tc.tile_pool(name="wpool", bufs=1) as wp,
            tc.tile_pool(name="xpool", bufs=1) as xp,
            tc.tile_pool(name="apool", bufs=1) as ap_,
        ):
            # ---------------- resident small tensors ----------------
            alpha = wp.tile([P, 1], F32)
            nc.vector.memset(alpha, 0.2)
            b2_sb = wp.tile([P, CT], F32)
            nc.sync.dma_start(out=b2_sb, in_=ctile(d_b2))
            b3_sb = wp.tile([P, CT], F32)
            nc.sync.dma_start(out=b3_sb, in_=ctile(d_b3))
            s4_sb = wp.tile([P, CT], F32)
            nc.sync.dma_start(out=s4_sb, in_=ctile(d_s4))
            b4p_sb = wp.tile([P, CT], F32)
            nc.sync.dma_start(out=b4p_sb, in_=ctile(d_b4p))
            bx1_sb = wp.tile([64, 1], F32)
            nc.sync.dma_start(out=bx1_sb, in_=d_bx1[:, None])
            bx2_sb = wp.tile([3, 1], F32)
            nc.sync.dma_start(out=bx2_sb, in_=d_bx2[:, None])
            b1_8_sb = wp.tile([P, CT], F32)
            nc.sync.dma_start(out=b1_8_sb, in_=ctile(d_b1_8))
            z_sb = wp.tile([P, CT], BF16)
            nc.sync.dma_start(out=z_sb, in_=ctile(d_z))

            # big weights (prefetched; used from layer 2 on)
            S_sb = wp.tile([P, NT, N], BF16)
            nc.sync.dma_start(out=S_sb, in_=d_S.rearrange("(t p) n -> p t n", p=P))
            W2T_sb = wp.tile([P, CT, C], BF16)
            nc.sync.dma_start(out=W2T_sb, in_=cmat(d_W2T))
            W3T_sb = wp.tile([P, CT, C], BF16)
            nc.sync.dma_start(out=W3T_sb, in_=cmat(d_W3T))
            W4T_sb = wp.tile([P, CT, C], BF16)
            nc.sync.dma_start(out=W4T_sb, in_=cmat(d_W4T))
            Wx1T_sb = wp.tile([P, CT, 64], BF16)
            nc.sync.dma_start(out=Wx1T_sb, in_=cmat(d_Wx1T))
            Wx2T_sb = wp.tile([64, 3], F32)
            nc.sync.dma_start(out=Wx2T_sb, in_=d_Wx2T[:, :])

            # X: current activation map, bf16, (128, c-tile, n)
            X = xp.tile([P, CT, N], BF16, tag="X")

            # ---------------- phase 0 + 1: u, then layer 1 ----------------
            G = 2048  # pairs per psum group (4 banks)
            with tc.tile_pool(name="l1pool", bufs=1) as l1p:
                W1zT_sb = l1p.tile([P, CT, C], BF16)
                nc.sync.dma_start(out=W1zT_sb, in_=cmat(d_W1zT))
                W6T_sb = l1p.tile([8, C], BF16)
                nc.sync.dma_start(out=W6T_sb, in_=d_W6T[:, :])
                btilde = l1p.tile([P, CT], F32)

                with (
                    tc.tile_pool(name="qpool", bufs=2) as qp,
                    tc.tile_pool(name="pairpool", bufs=2) as prp,
                    tc.tile_pool(name="psL1", bufs=2, space="PSUM") as ps1,
                ):
                    # u = W1z @ z; btilde = (u + b1)/8
                    for m in range(CT):
                        ps = ps1.tile([P, G], F32, tag="ps1")
                        for k in range(CT):
                            nc.tensor.matmul(
                                ps[:, 0:1],
                                W1zT_sb[:, k, m * P : (m + 1) * P],
                                z_sb[:, k, None],
                                start=(k == 0),
                                stop=(k == CT - 1),
                            )
                        nc.scalar.activation(
                            btilde[:, m, None], ps[:, 0:1], AF.Identity,
                            bias=b1_8_sb[:, m, None], scale=0.125,
                        )

                    # layer 1: h = W6' @ Q (+btilde), groups of G pairs
                    for g in range(NK // G):
                        q_sb = qp.tile([8, G], BF16, tag="q")
                        nc.sync.dma_start(
                            out=q_sb, in_=d_Q[:, g * G : (g + 1) * G]
                        )
                        for m in range(CT):
                            ps = ps1.tile([P, G], F32, tag="ps1")
                            for c in range(G // 512):
                                nc.tensor.matmul(
                                    ps[:, c * 512 : (c + 1) * 512],
                                    W6T_sb[:, m * P : (m + 1) * P],
                                    q_sb[:, c * 512 : (c + 1) * 512],
                                    start=True,
                                    stop=True,
                                )
                            pair = prp.tile([P, G // K, K], BF16, tag="pair")
                            nc.scalar.activation(
                                pair.rearrange("p n k -> p (n k)"), ps,
                                AF.Prelu, bias=btilde[:, m, None],
                                scale=1.0, alpha=alpha[:, :],
                            )
                            # tree-reduce sum over k (1/8 folded into W6/btilde)
                            t1 = prp.tile([P, G // K, 4], BF16, tag="t1")
                            nc.vector.tensor_add(
                                t1, pair[:, :, 0:4], pair[:, :, 4:8]
                            )
                            t2 = prp.tile([P, G // K, 2], BF16, tag="t2")
                            nc.vector.tensor_add(t2, t1[:, :, 0:2], t1[:, :, 2:4])
                            nc.vector.tensor_add(
                                X[:, m, g * (G // K) : (g + 1) * (G // K)],
                                t2[:, :, 0],
                                t2[:, :, 1],
                            )

            # ---------------- layers 2..4 ----------------
            psA_ctx = tc.tile_pool(name="psA", bufs=8, space="PSUM")
            psA = psA_ctx.__enter__()
            A_bf = ap_.tile([P, CT, N], BF16, tag="A")
            AT_bf = ap_.tile([P, NT, C], BF16, tag="AT")

            layer_params = [
                (W2T_sb, b2_sb, None),
                (W3T_sb, b3_sb, None),
                (W4T_sb, b4p_sb, s4_sb),
            ]
            for li, (WT_sb, bias_sb, scale_sb) in enumerate(layer_params):
                last = li == len(layer_params) - 1
                # H = W @ X ; A = prelu(scale*H + bias)
                for m in range(CT):
                    pss = []
                    for _i in range(4):
                        ps_i = psA.tile([P, 512], F32, tag="psA", name=f"psA_{_i}")
                        pss.append(ps_i)
                    for k in range(CT):
                        for nch in range(4):
                            nc.tensor.matmul(
                                pss[nch],
                                WT_sb[:, k, m * P : (m + 1) * P],
                                X[:, k, nch * 512 : (nch + 1) * 512],
                                start=(k == 0),
                                stop=(k == CT - 1),
                            )
                    for nch in range(4):
                        nc.scalar.activation(
                            A_bf[:, m, nch * 512 : (nch + 1) * 512], pss[nch],
                            AF.Prelu, bias=bias_sb[:, m, None],
                            scale=(scale_sb[:, m, None] if scale_sb is not None else 1.0),
                            alpha=alpha[:, :],
                        )
                # AT = A^T via DMA transpose (bf16)
                for m in range(CT):
                    nc.sync.dma_start(
                        out=AT_bf[:, :, m * P : (m + 1) * P],
                        in_=A_bf[:, m, :],
                        transpose=True,
                    )
                # X' = A @ S   (lhsT = AT blocks, rhs = S)
                for m in range(CT):
                    pss = []
                    for _i in range(4):
                        ps_i = psA.tile([P, 512], F32, tag="psA", name=f"psA_{_i}")
                        pss.append(ps_i)
                    for k in range(NT):
                        for nch in range(4):
                            nc.tensor.matmul(
                                pss[nch],
                                AT_bf[:, k, m * P : (m + 1) * P],
                                S_sb[:, k, nch * 512 : (nch + 1) * 512],
                                start=(k == 0),
                                stop=(k == NT - 1),
                            )
                    for nch in range(4):
                        nc.any.tensor_copy(
                            X[:, m, nch * 512 : (nch + 1) * 512], pss[nch]
                        )
                        if last:
                            # fp32 feature output, cast during SWDGE DMA
                            nc.gpsimd.dma_start(
                                out=d_feat.rearrange("(t p) n -> p t n", p=P)[
                                    :, m, nch * 512 : (nch + 1) * 512
                                ],
                                in_=X[:, m, nch * 512 : (nch + 1) * 512],
                            )

            # ---------------- final mlp ----------------
            t5 = ap_.tile([64, N], F32, tag="t5")
            for nch in range(4):
                ps = psA.tile([P, 512], F32, tag="psA")
                for k in range(CT):
                    nc.tensor.matmul(
                        ps[:64, :],
                        Wx1T_sb[:, k, :],
                        X[:, k, nch * 512 : (nch + 1) * 512],
                        start=(k == 0),
                        stop=(k == CT - 1),
                    )
                nc.scalar.activation(
                    t5[:, nch * 512 : (nch + 1) * 512], ps[:64, :],
                    AF.Prelu, bias=bx1_sb[:, :], scale=1.0, alpha=alpha[:64, :],
                )
            sig = ap_.tile([3, N], F32, tag="sig")
            for nch in range(4):
                ps = psA.tile([P, 512], F32, tag="psA")
                nc.tensor.matmul(
                    ps[:3, :],
                    Wx2T_sb[:, :],
                    t5[:, nch * 512 : (nch + 1) * 512],
                    start=True,
                    stop=True,
                )
                nc.scalar.activation(
                    sig[:, nch * 512 : (nch + 1) * 512], ps[:3, :],
                    AF.Sigmoid, bias=bx2_sb[:, :], scale=1.0,
                )
            pcs_sb = ap_.tile([3, N], F32, tag="pcs")
            nc.vector.tensor_scalar(
                out=pcs_sb, in0=sig, scalar1=-0.5, scalar2=None, op0=ALU.add
            )
            nc.sync.dma_start(out=d_pcs[:, :], in_=pcs_sb)
            psA_ctx.__exit__(None, None, None)

    return nc


def _get_program():
    if "nc" not in _CACHED:
        _CACHED["nc"] = _build_program()
    return _CACHED["nc"]


def _prep_inputs(z, sphere, knn_idx, W1, b1, W2, b2, W3, b3, W4, b4,
                 bn_g, bn_b, bn_m, bn_v, Wx1, bx1, Wx2, bx2):
    bf = lambda a: np.ascontiguousarray(np.asarray(a, np.float32).astype(ml_dtypes.bfloat16))
    f32 = lambda a: np.ascontiguousarray(np.asarray(a, np.float32))

    # averaging matrix S[m, n] = count(idx[n, :] == m) / 8
    S = np.zeros((N, N), np.float32)
    np.add.at(S, (np.asarray(knn_idx, np.int64), np.arange(N)[:, None]), 1.0 / K)

    xyz = np.asarray(sphere, np.float32).T                  # (3, N)
    xyzG = xyz[:, np.asarray(knn_idx, np.int64)].reshape(3, NK)
    xyzRep = np.repeat(xyz, K, axis=1)
    Q = np.zeros((8, NK), np.float32)
    Q[0:3] = xyzRep
    Q[3:6] = xyzG

    W1 = np.asarray(W1, np.float32)
    W1z = W1[:, 6:518]
    W6 = np.zeros((C, 8), np.float32)
    W6[:, 0:3] = W1[:, 0:3] - W1[:, 3:6]
    W6[:, 3:6] = W1[:, 3:6]
    W6_8 = W6 / K                                           # fold mean 1/8

    s4 = np.asarray(bn_g, np.float32) / np.sqrt(np.asarray(bn_v, np.float32) + BN_EPS)
    t4 = np.asarray(bn_b, np.float32) - np.asarray(bn_m, np.float32) * s4
    b4p = s4 * np.asarray(b4, np.float32) + t4

    common = {
        "S": bf(S),
        "Q": bf(Q),
        "W6T": bf(W6_8.T),
        "W1zT": bf(np.asarray(W1z, np.float32).T),
        "W2T": bf(np.asarray(W2, np.float32).T),
        "W3T": bf(np.asarray(W3, np.float32).T),
        "W4T": bf(np.asarray(W4, np.float32).T),
        "Wx1T": bf(np.asarray(Wx1, np.float32).T),
        "Wx2T": f32(np.asarray(Wx2, np.float32).T),
        "b1_8": f32(np.asarray(b1, np.float32) / K),
        "b2": f32(b2),
        "b3": f32(b3),
        "s4": f32(s4),
        "b4p": f32(b4p),
        "bx1": f32(bx1),
        "bx2": f32(bx2),
    }
    z = np.asarray(z, np.float32)
    return [dict(common, z=bf(z[i])) for i in range(z.shape[0])]


def kernel(z, sphere, knn_idx, W1, b1, W2, b2, W3, b3, W4, b4,
           bn_g, bn_b, bn_m, bn_v, Wx1, bx1, Wx2, bx2, point_num):
    B = np.asarray(z).shape[0]
    assert B == 8 and np.asarray(knn_idx).shape == (N, K)

    in_maps = _prep_inputs(z, sphere, knn_idx, W1, b1, W2, b2, W3, b3, W4, b4,
                           bn_g, bn_b, bn_m, bn_v, Wx1, bx1, Wx2, bx2)
    _CACHED["in_maps"] = in_maps

    nc = _get_program()
    res = run_bass_kernel_spmd(nc, in_maps, core_ids=list(range(B)))

    feature = np.stack([res.results[i]["feature"] for i in range(B)], 0)  # (B, C, N)
    pcs = np.stack([res.results[i]["pcs"] for i in range(B)], 0)          # (B, 3, N)
    pcs = np.transpose(pcs, (0, 2, 1)).astype(np.float32)                 # (B, N, 3)
    return (pcs, np.asarray(feature, np.float32))


# revision 8
# speedup vs baseline: 1.0859x; 1.0859x over previous
"""AtlasNet sphere EdgeConv generator — Trainium2 Bass kernel (8-core SPMD).

Math (per batch element b, all on one NeuronCore; B=8 across 8 cores):

  EdgeConv layer (conv1x1 -> lrelu -> mean over K gathered neighbors):
     out[:, n] = mean_k lrelu(W @ x[:, idx[n,k]] + b)
  conv1x1 is per-column and lrelu commutes with gather, so
     out = lrelu(W @ x + b) @ S
  where S[m, n] = #{k : idx[n,k] = m} / K — a dense (N, N) averaging matrix
  (entries k/8 are exact in bf16) applied as a PE matmul.

  Layer 1: f1 = [center; nbr-center; z] with W1 = [W1c | W1r | W1z]:
     h[:, n, k] = W6 @ [xyz[:, n]; xyz[:, idx[n,k]]] + (W1z @ z + b1)
  with W6 = [W1c - W1r | W1r] — a rank-6 contraction.  The per-batch bias
  u = W1z @ z / 8 is computed on-device as a row matmul (lhsT = z column)
  and written into a spare contraction row of the layer-1 weights, with the
  matching Q row set to 1, so the pair-domain drain is a pure leaky-relu.
     out1 = sum_k lrelu(h/8)    (lrelu is positively homogeneous)

  Layer 4 batch-norm folds into a per-channel scale/bias on the activation.
"""

import sys
import types

sys.path.insert(0, "/opt/trn_rl_repo")

import numpy as np
import ml_dtypes

import bass_rust
import concourse.bass as bass
import concourse.mybir as mybir
import concourse.tile as tile
from concourse.bass_utils import run_bass_kernel_spmd

F32 = mybir.dt.float32
BF16 = mybir.dt.bfloat16
AF = mybir.ActivationFunctionType
ALU = mybir.AluOpType

N = 2048          # points
K = 8             # knn
C = 512           # channels
NK = N * K        # pair count
P = 128           # partitions
CT = C // P       # channel tiles (4)
NT = N // P       # point tiles (16)
BN_EPS = 1e-5

_CACHED = {}


# ---------------------------------------------------------------------------
# Workaround for this walrus build's 'Too many sync wait commands' limit:
# at most 1 sync wait per instruction (2 for InstEventSemaphore).  Tile's
# rust sem-assignment attaches more, so (a) emit the final drain waits as
# individual wait instructions with exact final sem values, and (b) spill
# excess waits from any instruction onto same-engine EventSemaphore insts.
# ---------------------------------------------------------------------------
def _iter_blocks(nc):
    for fn in nc.m.functions:
        for bb in fn.blocks:
            yield bb


def _sem_totals(nc):
    totals = {}
    for bb in _iter_blocks(nc):
        for inst in bb.instructions:
            si = inst.sync_info
            if si is None:
                continue
            for u in si.on_update:
                v = getattr(u, "value", None)
                if v is None:
                    continue
                totals[u.id] = totals.get(u.id, 0) + v
    return totals


def _legalize_waits(nc):
    def make_evsem(engine, waits):
        with nc.semaphore() as tmp_sem:
            bi = nc.engines[engine].wait_ge(tmp_sem, 0)
        inst = bi.ins
        cur = nc.cur_bb.bb
        lst = cur.instructions
        assert lst and lst[-1].name == inst.name
        cur.instructions = lst[:-1]
        inst.sync_info = bass_rust.SyncInfo(on_wait=list(waits), on_update=[])
        return inst

    for bb in _iter_blocks(nc):
        lst = bb.instructions
        changed = False
        out = []
        for inst in lst:
            si = inst.sync_info
            waits = list(si.on_wait) if si is not None else []
            cap = 2 if isinstance(inst, mybir.InstEventSemaphore) else 1
            if len(waits) > cap:
                spill, keep = waits[:-cap], waits[-cap:]
                for i in range(0, len(spill), 2):
                    out.append(make_evsem(inst.engine, spill[i : i + 2]))
                inst.sync_info = bass_rust.SyncInfo(
                    on_wait=keep, on_update=list(si.on_update)
                )
                changed = True
            out.append(inst)
        if changed:
            bb.instructions = out


def _patched_drain_and_barrier(self, tick_clock, wait_clock):
    nc = self.nc
    totals = _sem_totals(nc)
    allocated = wait_clock.sems.allocated()
    gc = tick_clock.global_clock
    for proc, sem in allocated.items():
        tick = gc[proc]
        target = totals.get(sem.num, 0)
        if tick > 0 and target > 0:
            nc.sync.wait_ge(sem, target)
    nc.sync.drain()
    nc.all_engine_barrier()
    popped = nc._tile_sem_poison_stack.pop()
    assert popped is self._sem_poison
    nc.clear_and_free_semaphores(list(self.sems.allocated().values()))
    nc.all_engine_barrier()
    _legalize_waits(nc)


tile.TileContext._drain_and_barrier = _patched_drain_and_barrier


# ---------------------------------------------------------------------------
# device program
# ---------------------------------------------------------------------------
def _build_program():
    nc = bass.Bass()

    # all big tensors pre-tiled on host to (128, ...) per-partition-contiguous
    d_S = nc.declare_dram_parameter("S", [P, NT, N], BF16, isOutput=False)
    d_Q = nc.declare_dram_parameter("Q", [8, NK], BF16, isOutput=False)
    d_W6T = nc.declare_dram_parameter("W6T", [8, C], BF16, isOutput=False)
    d_W1zT = nc.declare_dram_parameter("W1zT", [P, CT, C], BF16, isOutput=False)
    d_W2T = nc.declare_dram_parameter("W2T", [P, CT, C], BF16, isOutput=False)
    d_W3T = nc.declare_dram_parameter("W3T", [P, CT, C], BF16, isOutput=False)
    d_W4T = nc.declare_dram_parameter("W4T", [P, CT, C], BF16, isOutput=False)
    d_Wx1T = nc.declare_dram_parameter("Wx1T", [P, CT, 64], BF16, isOutput=False)
    # bpack columns: 4:8 b2 | 8:12 b3 | 12:16 s4 | 16:20 b4p
    #   | 20:24 z(f32) | 24 bx1 (p<64) | 25 bx2 (p<3) | 26:29 Wx2T (p<64)
    d_bpack = nc.declare_dram_parameter("bpack", [P, 29], F32, isOutput=False)

    d_feat = nc.declare_dram_parameter("feature", [C, N], F32, isOutput=True)
    d_pcs = nc.declare_dram_parameter("pcs", [3, N], F32, isOutput=True)

    with tile.TileContext(nc) as tc:
        with (
            tc.tile_pool(name="wpool", bufs=1) as wp,
            tc.tile_pool(name="xpool", bufs=1) as xp,
            tc.tile_pool(name="apool", bufs=1) as ap_,
        ):
            # ---------------- resident small tensors ----------------
            alpha = wp.tile([P, 1], F32)
            nc.vector.memset(alpha, 0.2)
            bpack = wp.tile([P, 29], F32)
            nc.sync.dma_start(out=bpack, in_=d_bpack[:, :])
            b2_sb = bpack[:, 4:8]
            b3_sb = bpack[:, 8:12]
            s4_sb = bpack[:, 12:16]
            b4p_sb = bpack[:, 16:20]
            bx1_sb = bpack[:64, 24:25]
            bx2_sb = bpack[:3, 25:26]
            Wx2T_sb = bpack[:64, 26:29]
            z_sb = wp.tile([P, CT], BF16)
            nc.vector.tensor_copy(z_sb, bpack[:, 20:24])

            # big weights: SWDGE queue so they overlap with layer 1; S last
            W2T_sb = wp.tile([P, CT, C], BF16)
            nc.gpsimd.dma_start(out=W2T_sb, in_=d_W2T[:, :, :])
            W3T_sb = wp.tile([P, CT, C], BF16)
            nc.gpsimd.dma_start(out=W3T_sb, in_=d_W3T[:, :, :])
            W4T_sb = wp.tile([P, CT, C], BF16)
            nc.gpsimd.dma_start(out=W4T_sb, in_=d_W4T[:, :, :])
            Wx1T_sb = wp.tile([P, CT, 64], BF16)
            nc.gpsimd.dma_start(out=Wx1T_sb, in_=d_Wx1T[:, :, :])
            S_sb = wp.tile([P, NT, N], BF16)
            nc.gpsimd.dma_start(out=S_sb, in_=d_S[:, :, :])

            # X: current activation map, bf16, (128, c-tile, n)
            X = xp.tile([P, CT, N], BF16, tag="X")

            # ---------------- layer 1 (incl. on-device u row) ----------------
            G = 2048  # pairs per psum group (4 banks)
            with tc.tile_pool(name="l1pool", bufs=1) as l1p:
                W1zT_sb = l1p.tile([P, CT, C], BF16)
                nc.scalar.dma_start(out=W1zT_sb, in_=d_W1zT[:, :, :])
                # W6T rows: 0 = u/8 (device), 1 = b1/8 (host), 2..7 = W6'/8
                W6T_sb = l1p.tile([8, C], BF16)
                nc.scalar.dma_start(out=W6T_sb, in_=d_W6T[:, :])

                with (
                    tc.tile_pool(name="qpool", bufs=2) as qp,
                    tc.tile_pool(name="pairpool", bufs=2) as prp,
                    tc.tile_pool(name="psL1", bufs=2, space="PSUM") as ps1,
                ):
                    # u row: uT = zT @ W1zT (1 x C), scaled by 1/8 into W6T row 0
                    ps_u = ps1.tile([P, G], F32, tag="ps1")
                    for k in range(CT):
                        nc.tensor.matmul(
                            ps_u[:1, 0:C],
                            z_sb[:, k, None],
                            W1zT_sb[:, k, :],
                            start=(k == 0),
                            stop=(k == CT - 1),
                        )
                    nc.scalar.activation(
                        W6T_sb[0:1, :], ps_u[:1, 0:C], AF.Copy, scale=0.125
                    )

                    # layer 1: h = W6' @ Q (bias rows folded in), G-pair groups
                    for g in range(NK // G):
                        q_sb = qp.tile([8, G], BF16, tag="q")
                        nc.sync.dma_start(
                            out=q_sb, in_=d_Q[:, g * G : (g + 1) * G]
                        )
                        for m in range(CT):
                            ps = ps1.tile([P, G], F32, tag="ps1")
                            for c in range(G // 512):
                                nc.tensor.matmul(
                                    ps[:, c * 512 : (c + 1) * 512],
                                    W6T_sb[:, m * P : (m + 1) * P],
                                    q_sb[:, c * 512 : (c + 1) * 512],
                                    start=True,
                                    stop=True,
                                )
                            pair = prp.tile([P, G // K, K], BF16, tag="pair")
                            nc.scalar.activation(
                                pair.rearrange("p n k -> p (n k)"), ps,
                                AF.Prelu, bias=0.0, scale=1.0, alpha=alpha[:, :],
                            )
                            # tree-reduce sum over k (1/8 folded into weights)
                            t1 = prp.tile([P, G // K, 4], BF16, tag="t1")
                            nc.vector.tensor_add(
                                t1, pair[:, :, 0:4], pair[:, :, 4:8]
                            )
                            t2 = prp.tile([P, G // K, 2], BF16, tag="t2")
                            nc.vector.tensor_add(t2, t1[:, :, 0:2], t1[:, :, 2:4])
                            nc.vector.tensor_add(
                                X[:, m, g * (G // K) : (g + 1) * (G // K)],
                                t2[:, :, 0],
                                t2[:, :, 1],
                            )

            # ---------------- layers 2..4 ----------------
            psA_ctx = tc.tile_pool(name="psA", bufs=2, space="PSUM")
            psA = psA_ctx.__enter__()
            A_bf = ap_.tile([P, CT, N], BF16, tag="A")
            AT_bf = ap_.tile([P, NT, C], BF16, tag="AT")

            layer_params = [
                (W2T_sb, b2_sb, None),
                (W3T_sb, b3_sb, None),
                (W4T_sb, b4p_sb, s4_sb),
            ]
            NW = 512  # moving-operand width per matmul
            for li, (WT_sb, bias_sb, scale_sb) in enumerate(layer_params):
                last = li == len(layer_params) - 1
                # H = W @ X ; A = prelu(scale*H + bias)
                for m in range(CT):
                    ps = psA.tile([P, N], F32, tag="psA", name="psW")
                    for k in range(CT):
                        for nch in range(N // NW):
                            nc.tensor.matmul(
                                ps[:, nch * NW : (nch + 1) * NW],
                                WT_sb[:, k, m * P : (m + 1) * P],
                                X[:, k, nch * NW : (nch + 1) * NW],
                                start=(k == 0),
                                stop=(k == CT - 1),
                            )
                    nc.scalar.activation(
                        A_bf[:, m, :], ps,
                        AF.Prelu, bias=bias_sb[:, m, None],
                        scale=(scale_sb[:, m, None] if scale_sb is not None else 1.0),
                        alpha=alpha[:, :],
                    )
                # AT = A^T via DMA transpose (bf16)
                for m in range(CT):
                    nc.sync.dma_start(
                        out=AT_bf[:, :, m * P : (m + 1) * P],
                        in_=A_bf[:, m, :],
                        transpose=True,
                    )
                # X' = A @ S   (lhsT = AT blocks, rhs = S)
                for m in range(CT):
                    ps = psA.tile([P, N], F32, tag="psA", name="psS")
                    for k in range(NT):
                        for nch in range(N // NW):
                            nc.tensor.matmul(
                                ps[:, nch * NW : (nch + 1) * NW],
                                AT_bf[:, k, m * P : (m + 1) * P],
                                S_sb[:, k, nch * NW : (nch + 1) * NW],
                                start=(k == 0),
                                stop=(k == NT - 1),
                            )
                    nc.any.tensor_copy(X[:, m, :], ps)
                    if last:
                        # fp32 feature output, cast during SWDGE DMA
                        nc.gpsimd.dma_start(
                            out=d_feat.rearrange("(t p) n -> p t n", p=P)[:, m, :],
                            in_=X[:, m, :],
                        )

            # ---------------- final mlp ----------------
            t5 = ap_.tile([64, N], F32, tag="t5")
            for nch in range(4):
                ps = psA.tile([P, N], F32, tag="psA", name="psM")
                for k in range(CT):
                    nc.tensor.matmul(
                        ps[:64, 0:512],
                        Wx1T_sb[:, k, :],
                        X[:, k, nch * 512 : (nch + 1) * 512],
                        start=(k == 0),
                        stop=(k == CT - 1),
                    )
                nc.scalar.activation(
                    t5[:, nch * 512 : (nch + 1) * 512], ps[:64, 0:512],
                    AF.Prelu, bias=bx1_sb, scale=1.0, alpha=alpha[:64, :],
                )
            sig = ap_.tile([3, N], F32, tag="sig")
            for nch in range(4):
                ps = psA.tile([P, N], F32, tag="psA", name="psM2")
                nc.tensor.matmul(
                    ps[:3, 0:512],
                    Wx2T_sb,
                    t5[:, nch * 512 : (nch + 1) * 512],
                    start=True,
                    stop=True,
                )
                nc.scalar.activation(
                    sig[:, nch * 512 : (nch + 1) * 512], ps[:3, 0:512],
                    AF.Sigmoid, bias=bx2_sb, scale=1.0,
                )
            pcs_sb = ap_.tile([3, N], F32, tag="pcs")
            nc.vector.tensor_scalar(
                out=pcs_sb, in0=sig, scalar1=-0.5, scalar2=None, op0=ALU.add
            )
            nc.sync.dma_start(out=d_pcs[:, :], in_=pcs_sb)
            psA_ctx.__exit__(None, None, None)

    return nc


def _get_program():
    if "nc" not in _CACHED:
        _CACHED["nc"] = _build_program()
    return _CACHED["nc"]


def _prep_inputs(z, sphere, knn_idx, W1, b1, W2, b2, W3, b3, W4, b4,
                 bn_g, bn_b, bn_m, bn_v, Wx1, bx1, Wx2, bx2):
    bf = lambda a: np.ascontiguousarray(
        np.asarray(a, np.float32).astype(ml_dtypes.bfloat16)
    )
    ptile = lambda a: np.ascontiguousarray(
        np.asarray(a, np.float32).reshape(-1, P, a.shape[-1]).transpose(1, 0, 2)
    )  # (T*P, M) -> (P, T, M)
    cvec = lambda a: np.asarray(a, np.float32).reshape(CT, P).T  # (C,) -> (P, CT)

    # averaging matrix S[m, n] = count(idx[n, :] == m) / 8
    idx = np.asarray(knn_idx, np.int64)
    S = np.zeros((N, N), np.float32)
    np.add.at(S, (idx, np.arange(N)[:, None]), 1.0 / K)

    xyz = np.asarray(sphere, np.float32).T                  # (3, N)
    Q = np.zeros((8, NK), np.float32)
    Q[0:2] = 1.0                                            # u and b1 rows
    Q[2:5] = np.repeat(xyz, K, axis=1)
    Q[5:8] = xyz[:, idx].reshape(3, NK)

    W1 = np.asarray(W1, np.float32)
    W1z = W1[:, 6:518]
    # rows: 0 = u/8 (device-filled), 1 = b1/8, 2:5 = (W1c-W1r)/8, 5:8 = W1r/8
    W6 = np.zeros((8, C), np.float32)
    W6[1] = np.asarray(b1, np.float32) / K
    W6[2:5] = (W1[:, 0:3] - W1[:, 3:6]).T / K
    W6[5:8] = W1[:, 3:6].T / K

    s4 = np.asarray(bn_g, np.float32) / np.sqrt(np.asarray(bn_v, np.float32) + BN_EPS)
    t4 = np.asarray(bn_b, np.float32) - np.asarray(bn_m, np.float32) * s4
    b4p = s4 * np.asarray(b4, np.float32) + t4

    z = np.asarray(z, np.float32)
    in_maps = []
    common = {
        "S": bf(ptile(S)),
        "Q": bf(Q),
        "W6T": bf(W6),
        "W1zT": bf(ptile(W1z.T)),
        "W2T": bf(ptile(np.asarray(W2, np.float32).T)),
        "W3T": bf(ptile(np.asarray(W3, np.float32).T)),
        "W4T": bf(ptile(np.asarray(W4, np.float32).T)),
        "Wx1T": bf(ptile(np.asarray(Wx1, np.float32).T)),
    }
    for i in range(z.shape[0]):
        bp = np.zeros((P, 29), np.float32)
        bp[:, 4:8] = cvec(b2)
        bp[:, 8:12] = cvec(b3)
        bp[:, 12:16] = cvec(s4)
        bp[:, 16:20] = cvec(b4p)
        bp[:, 20:24] = cvec(z[i])
        bp[:64, 24] = np.asarray(bx1, np.float32)
        bp[:3, 25] = np.asarray(bx2, np.float32)
        bp[:64, 26:29] = np.asarray(Wx2, np.float32).T
        in_maps.append(dict(common, bpack=np.ascontiguousarray(bp)))
    return in_maps


def kernel(z, sphere, knn_idx, W1, b1, W2, b2, W3, b3, W4, b4,
           bn_g, bn_b, bn_m, bn_v, Wx1, bx1, Wx2, bx2, point_num):
    B = np.asarray(z).shape[0]
    assert B == 8 and np.asarray(knn_idx).shape == (N, K)

    in_maps = _prep_inputs(z, sphere, knn_idx, W1, b1, W2, b2, W3, b3, W4, b4,
                           bn_g, bn_b, bn_m, bn_v, Wx1, bx1, Wx2, bx2)
    _CACHED["in_maps"] = in_maps

    nc = _get_program()
    res = run_bass_kernel_spmd(nc, in_maps, core_ids=list(range(B)))

    feature = np.stack([res.results[i]["feature"] for i in range(B)], 0)  # (B,C,N)
    pcs = np.stack([res.results[i]["pcs"] for i in range(B)], 0)          # (B,3,N)
    pcs = np.transpose(pcs, (0, 2, 1)).astype(np.float32)                 # (B,N,3)
    return (pcs, np.asarray(feature, np.float32))


# revision 9
# speedup vs baseline: 1.0900x; 1.0037x over previous
"""AtlasNet sphere EdgeConv generator — Trainium2 Bass kernel (8-core SPMD).

Math (per batch element b, all on one NeuronCore; B=8 across 8 cores):

  EdgeConv layer (conv1x1 -> lrelu -> mean over K gathered neighbors):
     out[:, n] = mean_k lrelu(W @ x[:, idx[n,k]] + b)
  conv1x1 is per-column and lrelu commutes with gather, so
     out = lrelu(W @ x + b) @ S
  where S[m, n] = #{k : idx[n,k] = m} / K — a dense (N, N) averaging matrix
  (entries k/8 are exact in bf16) applied as a PE matmul.

  Layer 1: f1 = [center; nbr-center; z] with W1 = [W1c | W1r | W1z]:
     h[:, n, k] = W6 @ [xyz[:, n]; xyz[:, idx[n,k]]] + (W1z @ z + b1)
  with W6 = [W1c - W1r | W1r] — a rank-6 contraction.  The per-batch bias
  u = W1z @ z / 8 is computed on-device as a row matmul (lhsT = z column)
  and written into a spare contraction row of the layer-1 weights, with the
  matching Q row set to 1, so the pair-domain drain is a pure leaky-relu.
     out1 = sum_k lrelu(h/8)    (lrelu is positively homogeneous)

  Layer 4 batch-norm folds into a per-channel scale/bias on the activation.
"""

import sys
import types

sys.path.insert(0, "/opt/trn_rl_repo")

import numpy as np
import ml_dtypes

import bass_rust
import concourse.bass as bass
import concourse.mybir as mybir
import concourse.tile as tile
from concourse.bass_utils import run_bass_kernel_spmd

F32 = mybir.dt.float32
BF16 = mybir.dt.bfloat16
AF = mybir.ActivationFunctionType
ALU = mybir.AluOpType

N = 2048          # points
K = 8             # knn
C = 512           # channels
NK = N * K        # pair count
P = 128           # partitions
CT = C // P       # channel tiles (4)
NT = N // P       # point tiles (16)
BN_EPS = 1e-5

_CACHED = {}


# ---------------------------------------------------------------------------
# Workaround for this walrus build's 'Too many sync wait commands' limit:
# at most 1 sync wait per instruction (2 for InstEventSemaphore).  Tile's
# rust sem-assignment attaches more, so (a) emit the final drain waits as
# individual wait instructions with exact final sem values, and (b) spill
# excess waits from any instruction onto same-engine EventSemaphore insts.
# ---------------------------------------------------------------------------
def _iter_blocks(nc):
    for fn in nc.m.functions:
        for bb in fn.blocks:
            yield bb


def _sem_totals(nc):
    totals = {}
    for bb in _iter_blocks(nc):
        for inst in bb.instructions:
            si = inst.sync_info
            if si is None:
                continue
            for u in si.on_update:
                v = getattr(u, "value", None)
                if v is None:
                    continue
                totals[u.id] = totals.get(u.id, 0) + v
    return totals


def _legalize_waits(nc):
    def make_evsem(engine, waits):
        with nc.semaphore() as tmp_sem:
            bi = nc.engines[engine].wait_ge(tmp_sem, 0)
        inst = bi.ins
        cur = nc.cur_bb.bb
        lst = cur.instructions
        assert lst and lst[-1].name == inst.name
        cur.instructions = lst[:-1]
        inst.sync_info = bass_rust.SyncInfo(on_wait=list(waits), on_update=[])
        return inst

    for bb in _iter_blocks(nc):
        lst = bb.instructions
        changed = False
        out = []
        for inst in lst:
            si = inst.sync_info
            waits = list(si.on_wait) if si is not None else []
            cap = 2 if isinstance(inst, mybir.InstEventSemaphore) else 1
            if len(waits) > cap:
                spill, keep = waits[:-cap], waits[-cap:]
                for i in range(0, len(spill), 2):
                    out.append(make_evsem(inst.engine, spill[i : i + 2]))
                inst.sync_info = bass_rust.SyncInfo(
                    on_wait=keep, on_update=list(si.on_update)
                )
                changed = True
            out.append(inst)
        if changed:
            bb.instructions = out


def _patched_drain_and_barrier(self, tick_clock, wait_clock):
    nc = self.nc
    totals = _sem_totals(nc)
    allocated = wait_clock.sems.allocated()
    gc = tick_clock.global_clock
    for proc, sem in allocated.items():
        tick = gc[proc]
        target = totals.get(sem.num, 0)
        if tick > 0 and target > 0:
            nc.sync.wait_ge(sem, target)
    nc.sync.drain()
    nc.all_engine_barrier()
    popped = nc._tile_sem_poison_stack.pop()
    assert popped is self._sem_poison
    nc.clear_and_free_semaphores(list(self.sems.allocated().values()))
    nc.all_engine_barrier()
    _legalize_waits(nc)


tile.TileContext._drain_and_barrier = _patched_drain_and_barrier


# ---------------------------------------------------------------------------
# device program
# ---------------------------------------------------------------------------
def _build_program():
    nc = bass.Bass()

    # all big tensors pre-tiled on host to (128, ...) per-partition-contiguous
    d_S = nc.declare_dram_parameter("S", [P, NT, N], BF16, isOutput=False)
    d_Q = nc.declare_dram_parameter("Q", [8, NK], BF16, isOutput=False)
    d_W6T = nc.declare_dram_parameter("W6T", [8, C], BF16, isOutput=False)
    d_W1zT = nc.declare_dram_parameter("W1zT", [P, CT, C], BF16, isOutput=False)
    d_W2T = nc.declare_dram_parameter("W2T", [P, CT, C], BF16, isOutput=False)
    d_W3T = nc.declare_dram_parameter("W3T", [P, CT, C], BF16, isOutput=False)
    d_W4T = nc.declare_dram_parameter("W4T", [P, CT, C], BF16, isOutput=False)
    d_Wx1T = nc.declare_dram_parameter("Wx1T", [P, CT, 64], BF16, isOutput=False)
    # bpack columns: 4:8 b2 | 8:12 b3 | 12:16 s4 | 16:20 b4p
    #   | 20:24 z(f32) | 24 bx1 (p<64) | 25 bx2 (p<3) | 26:29 Wx2T (p<64)
    d_bpack = nc.declare_dram_parameter("bpack", [P, 29], F32, isOutput=False)

    d_feat = nc.declare_dram_parameter("feature", [C, N], F32, isOutput=True)
    d_pcs = nc.declare_dram_parameter("pcs", [3, N], F32, isOutput=True)

    with tile.TileContext(nc) as tc:
        with (
            tc.tile_pool(name="wpool", bufs=1) as wp,
            tc.tile_pool(name="xpool", bufs=1) as xp,
            tc.tile_pool(name="apool", bufs=1) as ap_,
        ):
            # ---------------- resident small tensors ----------------
            alpha = wp.tile([P, 1], F32)
            nc.vector.memset(alpha, 0.2)
            bpack = wp.tile([P, 29], F32)
            nc.sync.dma_start(out=bpack, in_=d_bpack[:, :])
            b2_sb = bpack[:, 4:8]
            b3_sb = bpack[:, 8:12]
            s4_sb = bpack[:, 12:16]
            b4p_sb = bpack[:, 16:20]
            bx1_sb = bpack[:64, 24:25]
            bx2_sb = bpack[:3, 25:26]
            Wx2T_sb = bpack[:64, 26:29]
            z_sb = wp.tile([P, CT], BF16)
            nc.vector.tensor_copy(z_sb, bpack[:, 20:24])

            # big weights: SWDGE queue so they overlap with layer 1; S last
            W2T_sb = wp.tile([P, CT, C], BF16)
            nc.gpsimd.dma_start(out=W2T_sb, in_=d_W2T[:, :, :])
            W3T_sb = wp.tile([P, CT, C], BF16)
            nc.gpsimd.dma_start(out=W3T_sb, in_=d_W3T[:, :, :])
            W4T_sb = wp.tile([P, CT, C], BF16)
            nc.gpsimd.dma_start(out=W4T_sb, in_=d_W4T[:, :, :])
            Wx1T_sb = wp.tile([P, CT, 64], BF16)
            nc.gpsimd.dma_start(out=Wx1T_sb, in_=d_Wx1T[:, :, :])
            S_sb = wp.tile([P, NT, N], BF16)
            nc.gpsimd.dma_start(out=S_sb, in_=d_S[:, :, :])

            # X: current activation map, bf16, (128, c-tile, n)
            X = xp.tile([P, CT, N], BF16, tag="X")

            # ---------------- layer 1 (incl. on-device u row) ----------------
            G = 2048  # pairs per psum group (4 banks)
            with tc.tile_pool(name="l1pool", bufs=1) as l1p:
                W1zT_sb = l1p.tile([P, CT, C], BF16)
                nc.scalar.dma_start(out=W1zT_sb, in_=d_W1zT[:, :, :])
                # W6T rows: 0 = u/8 (device), 1 = b1/8 (host), 2..7 = W6'/8
                W6T_sb = l1p.tile([8, C], BF16)
                nc.scalar.dma_start(out=W6T_sb, in_=d_W6T[:, :])

                with (
                    tc.tile_pool(name="qpool", bufs=2) as qp,
                    tc.tile_pool(name="pairpool", bufs=2) as prp,
                    tc.tile_pool(name="psL1", bufs=2, space="PSUM") as ps1,
                ):
                    # u row: uT = zT @ W1zT (1 x C), scaled by 1/8 into W6T row 0
                    ps_u = ps1.tile([P, G], F32, tag="ps1")
                    for k in range(CT):
                        nc.tensor.matmul(
                            ps_u[:1, 0:C],
                            z_sb[:, k, None],
                            W1zT_sb[:, k, :],
                            start=(k == 0),
                            stop=(k == CT - 1),
                        )
                    nc.scalar.activation(
                        W6T_sb[0:1, :], ps_u[:1, 0:C], AF.Copy, scale=0.125
                    )

                    # layer 1: h = W6' @ Q (bias rows folded in), G-pair groups
                    for g in range(NK // G):
                        q_sb = qp.tile([8, G], BF16, tag="q")
                        nc.sync.dma_start(
                            out=q_sb, in_=d_Q[:, g * G : (g + 1) * G]
                        )
                        for m in range(CT):
                            ps = ps1.tile([P, G], F32, tag="ps1")
                            for c in range(G // 512):
                                nc.tensor.matmul(
                                    ps[:, c * 512 : (c + 1) * 512],
                                    W6T_sb[:, m * P : (m + 1) * P],
                                    q_sb[:, c * 512 : (c + 1) * 512],
                                    start=True,
                                    stop=True,
                                )
                            pair = prp.tile([P, G // K, K], BF16, tag="pair")
                            nc.scalar.activation(
                                pair.rearrange("p n k -> p (n k)"), ps,
                                AF.Prelu, bias=0.0, scale=1.0, alpha=alpha[:, :],
                            )
                            # tree-reduce sum over k (1/8 folded into weights)
                            t1 = prp.tile([P, G // K, 4], BF16, tag="t1")
                            nc.vector.tensor_add(
                                t1, pair[:, :, 0:4], pair[:, :, 4:8]
                            )
                            t2 = prp.tile([P, G // K, 2], BF16, tag="t2")
                            nc.vector.tensor_add(t2, t1[:, :, 0:2], t1[:, :, 2:4])
                            nc.vector.tensor_add(
                                X[:, m, g * (G // K) : (g + 1) * (G // K)],
                                t2[:, :, 0],
                                t2[:, :, 1],
                            )

            # ---------------- layers 2..4 ----------------
            psA_ctx = tc.tile_pool(name="psA", bufs=2, space="PSUM")
            psA = psA_ctx.__enter__()
            A_bf = ap_.tile([P, CT, N], BF16, tag="A")
            AT_bf = ap_.tile([P, NT, C], BF16, tag="AT")

            layer_params = [
                (W2T_sb, b2_sb, None),
                (W3T_sb, b3_sb, None),
                (W4T_sb, b4p_sb, s4_sb),
            ]
            NW = 512  # moving-operand width per matmul
            for li, (WT_sb, bias_sb, scale_sb) in enumerate(layer_params):
                last = li == len(layer_params) - 1
                # H = W @ X ; A = prelu(scale*H + bias)
                for m in range(CT):
                    ps = psA.tile([P, N], F32, tag="psA", name="psW")
                    for k in range(CT):
                        for nch in range(N // NW):
                            nc.tensor.matmul(
                                ps[:, nch * NW : (nch + 1) * NW],
                                WT_sb[:, k, m * P : (m + 1) * P],
                                X[:, k, nch * NW : (nch + 1) * NW],
                                start=(k == 0),
                                stop=(k == CT - 1),
                            )
                    nc.scalar.activation(
                        A_bf[:, m, :], ps,
                        AF.Prelu, bias=bias_sb[:, m, None],
                        scale=(scale_sb[:, m, None] if scale_sb is not None else 1.0),
                        alpha=alpha[:, :],
                    )
                # AT = A^T via DMA transpose (bf16)
                for m in range(CT):
                    nc.sync.dma_start(
                        out=AT_bf[:, :, m * P : (m + 1) * P],
                        in_=A_bf[:, m, :],
                        transpose=True,
                    )
                # X' = A @ S   (lhsT = AT blocks, rhs = S)
                for m in range(CT):
                    ps = psA.tile([P, N], F32, tag="psA", name="psS")
                    for k in range(NT):
                        for nch in range(N // NW):
                            nc.tensor.matmul(
                                ps[:, nch * NW : (nch + 1) * NW],
                                AT_bf[:, k, m * P : (m + 1) * P],
                                S_sb[:, k, nch * NW : (nch + 1) * NW],
                                start=(k == 0),
                                stop=(k == NT - 1),
                            )
                    nc.vector.tensor_copy(X[:, m, :], ps)
                    if last:
                        # fp32 feature output, cast during SWDGE DMA
                        nc.gpsimd.dma_start(
                            out=d_feat.rearrange("(t p) n -> p t n", p=P)[:, m, :],
                            in_=X[:, m, :],
                        )

            # ---------------- final mlp ----------------
            t5 = ap_.tile([64, N], F32, tag="t5")
            for nch in range(4):
                ps = psA.tile([P, N], F32, tag="psA", name="psM")
                for k in range(CT):
                    nc.tensor.matmul(
                        ps[:64, 0:512],
                        Wx1T_sb[:, k, :],
                        X[:, k, nch * 512 : (nch + 1) * 512],
                        start=(k == 0),
                        stop=(k == CT - 1),
                    )
                nc.scalar.activation(
                    t5[:, nch * 512 : (nch + 1) * 512], ps[:64, 0:512],
                    AF.Prelu, bias=bx1_sb, scale=1.0, alpha=alpha[:64, :],
                )
            sig = ap_.tile([3, N], F32, tag="sig")
            pcs_sb = ap_.tile([3, N], F32, tag="pcs")
            for nch in range(4):
                ps = psA.tile([P, N], F32, tag="psA", name="psM2")
                nc.tensor.matmul(
                    ps[:3, 0:512],
                    Wx2T_sb,
                    t5[:, nch * 512 : (nch + 1) * 512],
                    start=True,
                    stop=True,
                )
                nc.scalar.activation(
                    sig[:, nch * 512 : (nch + 1) * 512], ps[:3, 0:512],
                    AF.Sigmoid, bias=bx2_sb, scale=1.0,
                )
                nc.vector.tensor_scalar(
                    out=pcs_sb[:, nch * 512 : (nch + 1) * 512],
                    in0=sig[:, nch * 512 : (nch + 1) * 512],
                    scalar1=-0.5, scalar2=None, op0=ALU.add,
                )
                nc.sync.dma_start(
                    out=d_pcs[:, nch * 512 : (nch + 1) * 512],
                    in_=pcs_sb[:, nch * 512 : (nch + 1) * 512],
                )
            psA_ctx.__exit__(None, None, None)

    return nc


def _get_program():
    if "nc" not in _CACHED:
        _CACHED["nc"] = _build_program()
    return _CACHED["nc"]


def _prep_inputs(z, sphere, knn_idx, W1, b1, W2, b2, W3, b3, W4, b4,
                 bn_g, bn_b, bn_m, bn_v, Wx1, bx1, Wx2, bx2):
    bf = lambda a: np.ascontiguousarray(
        np.asarray(a, np.float32).astype(ml_dtypes.bfloat16)
    )
    ptile = lambda a: np.ascontiguousarray(
        np.asarray(a, np.float32).reshape(-1, P, a.shape[-1]).transpose(1, 0, 2)
    )  # (T*P, M) -> (P, T, M)
    cvec = lambda a: np.asarray(a, np.float32).reshape(CT, P).T  # (C,) -> (P, CT)

    # averaging matrix S[m, n] = count(idx[n, :] == m) / 8
    idx = np.asarray(knn_idx, np.int64)
    S = np.zeros((N, N), np.float32)
    np.add.at(S, (idx, np.arange(N)[:, None]), 1.0 / K)

    xyz = np.asarray(sphere, np.float32).T                  # (3, N)
    Q = np.zeros((8, NK), np.float32)
    Q[0:2] = 1.0                                            # u and b1 rows
    Q[2:5] = np.repeat(xyz, K, axis=1)
    Q[5:8] = xyz[:, idx].reshape(3, NK)

    W1 = np.asarray(W1, np.float32)
    W1z = W1[:, 6:518]
    # rows: 0 = u/8 (device-filled), 1 = b1/8, 2:5 = (W1c-W1r)/8, 5:8 = W1r/8
    W6 = np.zeros((8, C), np.float32)
    W6[1] = np.asarray(b1, np.float32) / K
    W6[2:5] = (W1[:, 0:3] - W1[:, 3:6]).T / K
    W6[5:8] = W1[:, 3:6].T / K

    s4 = np.asarray(bn_g, np.float32) / np.sqrt(np.asarray(bn_v, np.float32) + BN_EPS)
    t4 = np.asarray(bn_b, np.float32) - np.asarray(bn_m, np.float32) * s4
    b4p = s4 * np.asarray(b4, np.float32) + t4

    z = np.asarray(z, np.float32)
    in_maps = []
    common = {
        "S": bf(ptile(S)),
        "Q": bf(Q),
        "W6T": bf(W6),
        "W1zT": bf(ptile(W1z.T)),
        "W2T": bf(ptile(np.asarray(W2, np.float32).T)),
        "W3T": bf(ptile(np.asarray(W3, np.float32).T)),
        "W4T": bf(ptile(np.asarray(W4, np.float32).T)),
        "Wx1T": bf(ptile(np.asarray(Wx1, np.float32).T)),
    }
    for i in range(z.shape[0]):
        bp = np.zeros((P, 29), np.float32)
        bp[:, 4:8] = cvec(b2)
        bp[:, 8:12] = cvec(b3)
        bp[:, 12:16] = cvec(s4)
        bp[:, 16:20] = cvec(b4p)
        bp[:, 20:24] = cvec(z[i])
        bp[:64, 24] = np.asarray(bx1, np.float32)
        bp[:3, 25] = np.asarray(bx2, np.float32)
        bp[:64, 26:29] = np.asarray(Wx2, np.float32).T
        in_maps.append(dict(common, bpack=np.ascontiguousarray(bp)))
    return in_maps


def kernel(z, sphere, knn_idx, W1, b1, W2, b2, W3, b3, W4, b4,
           bn_g, bn_b, bn_m, bn_v, Wx1, bx1, Wx2, bx2, point_num):
    B = np.asarray(z).shape[0]
    assert B == 8 and np.asarray(knn_idx).shape == (N, K)

    in_maps = _prep_inputs(z, sphere, knn_idx, W1, b1, W2, b2, W3, b3, W4, b4,
                           bn_g, bn_b, bn_m, bn_v, Wx1, bx1, Wx2, bx2)
    _CACHED["in_maps"] = in_maps

    nc = _get_program()
    res = run_bass_kernel_spmd(nc, in_maps, core_ids=list(range(B)))

    feature = np.stack([res.results[i]["feature"] for i in range(B)], 0)  # (B,C,N)
    pcs = np.stack([res.results[i]["pcs"] for i in range(B)], 0)          # (B,3,N)
    pcs = np.transpose(pcs, (0, 2, 1)).astype(np.float32)                 # (B,N,3)
    return (pcs, np.asarray(feature, np.float32))


# revision 10
# speedup vs baseline: 1.0934x; 1.0032x over previous
"""AtlasNet sphere EdgeConv generator — Trainium2 Bass kernel (8-core SPMD).

Math (per batch element b, all on one NeuronCore; B=8 across 8 cores):

  EdgeConv layer (conv1x1 -> lrelu -> mean over K gathered neighbors):
     out[:, n] = mean_k lrelu(W @ x[:, idx[n,k]] + b)
  conv1x1 is per-column and lrelu commutes with gather, so
     out = lrelu(W @ x + b) @ S
  where S[m, n] = #{k : idx[n,k] = m} / K — a dense (N, N) averaging matrix
  (entries k/8 are exact in bf16) applied as a PE matmul.

  Layer 1: f1 = [center; nbr-center; z] with W1 = [W1c | W1r | W1z]:
     h[:, n, k] = W6 @ [xyz[:, n]; xyz[:, idx[n,k]]] + (W1z @ z + b1)
  with W6 = [W1c - W1r | W1r] — a rank-6 contraction.  The per-batch bias
  u = W1z @ z / 8 is computed on-device as a row matmul (lhsT = z column)
  and written into a spare contraction row of the layer-1 weights, with the
  matching Q row set to 1, so the pair-domain drain is a pure leaky-relu.
     out1 = sum_k lrelu(h/8)    (lrelu is positively homogeneous)

  Layer 4 batch-norm folds into a per-channel scale/bias on the activation.
"""

import sys
import types

sys.path.insert(0, "/opt/trn_rl_repo")

import numpy as np
import ml_dtypes

import bass_rust
import concourse.bass as bass
import concourse.mybir as mybir
import concourse.tile as tile
from concourse.bass_utils import run_bass_kernel_spmd

F32 = mybir.dt.float32
BF16 = mybir.dt.float16  # fp16: same width as bf16, 4x finer mantissa
AF = mybir.ActivationFunctionType
ALU = mybir.AluOpType

N = 2048          # points
K = 8             # knn
C = 512           # channels
NK = N * K        # pair count
P = 128           # partitions
CT = C // P       # channel tiles (4)
NT = N // P       # point tiles (16)
BN_EPS = 1e-5

_CACHED = {}


# ---------------------------------------------------------------------------
# Workaround for this walrus build's 'Too many sync wait commands' limit:
# at most 1 sync wait per instruction (2 for InstEventSemaphore).  Tile's
# rust sem-assignment attaches more, so (a) emit the final drain waits as
# individual wait instructions with exact final sem values, and (b) spill
# excess waits from any instruction onto same-engine EventSemaphore insts.
# ---------------------------------------------------------------------------
def _iter_blocks(nc):
    for fn in nc.m.functions:
        for bb in fn.blocks:
            yield bb


def _sem_totals(nc):
    totals = {}
    for bb in _iter_blocks(nc):
        for inst in bb.instructions:
            si = inst.sync_info
            if si is None:
                continue
            for u in si.on_update:
                v = getattr(u, "value", None)
                if v is None:
                    continue
                totals[u.id] = totals.get(u.id, 0) + v
    return totals


def _legalize_waits(nc):
    def make_evsem(engine, waits):
        with nc.semaphore() as tmp_sem:
            bi = nc.engines[engine].wait_ge(tmp_sem, 0)
        inst = bi.ins
        cur = nc.cur_bb.bb
        lst = cur.instructions
        assert lst and lst[-1].name == inst.name
        cur.instructions = lst[:-1]
        inst.sync_info = bass_rust.SyncInfo(on_wait=list(waits), on_update=[])
        return inst

    for bb in _iter_blocks(nc):
        lst = bb.instructions
        changed = False
        out = []
        for inst in lst:
            si = inst.sync_info
            waits = list(si.on_wait) if si is not None else []
            cap = 2 if isinstance(inst, mybir.InstEventSemaphore) else 1
            if len(waits) > cap:
                spill, keep = waits[:-cap], waits[-cap:]
                for i in range(0, len(spill), 2):
                    out.append(make_evsem(inst.engine, spill[i : i + 2]))
                inst.sync_info = bass_rust.SyncInfo(
                    on_wait=keep, on_update=list(si.on_update)
                )
                changed = True
            out.append(inst)
        if changed:
            bb.instructions = out


def _patched_drain_and_barrier(self, tick_clock, wait_clock):
    nc = self.nc
    totals = _sem_totals(nc)
    allocated = wait_clock.sems.allocated()
    gc = tick_clock.global_clock
    for proc, sem in allocated.items():
        tick = gc[proc]
        target = totals.get(sem.num, 0)
        if tick > 0 and target > 0:
            nc.sync.wait_ge(sem, target)
    nc.sync.drain()
    nc.all_engine_barrier()
    popped = nc._tile_sem_poison_stack.pop()
    assert popped is self._sem_poison
    nc.clear_and_free_semaphores(list(self.sems.allocated().values()))
    nc.all_engine_barrier()
    _legalize_waits(nc)


tile.TileContext._drain_and_barrier = _patched_drain_and_barrier


# ---------------------------------------------------------------------------
# device program
# ---------------------------------------------------------------------------
def _build_program():
    nc = bass.Bass()

    # all big tensors pre-tiled on host to (128, ...) per-partition-contiguous
    d_S = nc.declare_dram_parameter("S", [P, NT, N], BF16, isOutput=False)
    d_Q = nc.declare_dram_parameter("Q", [8, NK], BF16, isOutput=False)
    d_W6T = nc.declare_dram_parameter("W6T", [8, C], BF16, isOutput=False)
    d_W1zT = nc.declare_dram_parameter("W1zT", [P, CT, C], BF16, isOutput=False)
    d_W2T = nc.declare_dram_parameter("W2T", [P, CT, C], BF16, isOutput=False)
    d_W3T = nc.declare_dram_parameter("W3T", [P, CT, C], BF16, isOutput=False)
    d_W4T = nc.declare_dram_parameter("W4T", [P, CT, C], BF16, isOutput=False)
    d_Wx1T = nc.declare_dram_parameter("Wx1T", [P, CT, 64], BF16, isOutput=False)
    # bpack columns: 4:8 b2 | 8:12 b3 | 12:16 s4 | 16:20 b4p
    #   | 20:24 z(f32) | 24 bx1 (p<64) | 25 bx2 (p<3) | 26:29 Wx2T (p<64)
    d_bpack = nc.declare_dram_parameter("bpack", [P, 29], F32, isOutput=False)

    d_feat = nc.declare_dram_parameter("feature", [C, N], F32, isOutput=True)
    d_pcs = nc.declare_dram_parameter("pcs", [3, N], F32, isOutput=True)

    with tile.TileContext(nc) as tc:
        with (
            tc.tile_pool(name="wpool", bufs=1) as wp,
            tc.tile_pool(name="xpool", bufs=1) as xp,
            tc.tile_pool(name="apool", bufs=1) as ap_,
        ):
            # ---------------- resident small tensors ----------------
            alpha = wp.tile([P, 1], F32)
            nc.vector.memset(alpha, 0.2)
            bpack = wp.tile([P, 29], F32)
            nc.sync.dma_start(out=bpack, in_=d_bpack[:, :])
            b2_sb = bpack[:, 4:8]
            b3_sb = bpack[:, 8:12]
            s4_sb = bpack[:, 12:16]
            b4p_sb = bpack[:, 16:20]
            bx1_sb = bpack[:64, 24:25]
            bx2_sb = bpack[:3, 25:26]
            Wx2T_sb = bpack[:64, 26:29]
            z_sb = wp.tile([P, CT], BF16)
            nc.vector.tensor_copy(z_sb, bpack[:, 20:24])

            # big weights: SWDGE queue so they overlap with layer 1; S last
            W2T_sb = wp.tile([P, CT, C], BF16)
            nc.gpsimd.dma_start(out=W2T_sb, in_=d_W2T[:, :, :])
            W3T_sb = wp.tile([P, CT, C], BF16)
            nc.gpsimd.dma_start(out=W3T_sb, in_=d_W3T[:, :, :])
            W4T_sb = wp.tile([P, CT, C], BF16)
            nc.gpsimd.dma_start(out=W4T_sb, in_=d_W4T[:, :, :])
            Wx1T_sb = wp.tile([P, CT, 64], BF16)
            nc.gpsimd.dma_start(out=Wx1T_sb, in_=d_Wx1T[:, :, :])
            S_sb = wp.tile([P, NT, N], BF16)
            nc.gpsimd.dma_start(out=S_sb, in_=d_S[:, :, :])

            # X: current activation map, bf16, (128, c-tile, n)
            X = xp.tile([P, CT, N], BF16, tag="X")

            # ---------------- layer 1 (incl. on-device u row) ----------------
            G = 2048  # pairs per psum group (4 banks)
            with tc.tile_pool(name="l1pool", bufs=1) as l1p:
                W1zT_sb = l1p.tile([P, CT, C], BF16)
                nc.scalar.dma_start(out=W1zT_sb, in_=d_W1zT[:, :, :])
                # W6T rows: 0 = u/8 (device), 1 = b1/8 (host), 2..7 = W6'/8
                W6T_sb = l1p.tile([8, C], BF16)
                nc.scalar.dma_start(out=W6T_sb, in_=d_W6T[:, :])

                with (
                    tc.tile_pool(name="qpool", bufs=2) as qp,
                    tc.tile_pool(name="pairpool", bufs=2) as prp,
                    tc.tile_pool(name="psL1", bufs=2, space="PSUM") as ps1,
                ):
                    # u row: uT = zT @ W1zT (1 x C), scaled by 1/8 into W6T row 0
                    ps_u = ps1.tile([P, G], F32, tag="ps1")
                    for k in range(CT):
                        nc.tensor.matmul(
                            ps_u[:1, 0:C],
                            z_sb[:, k, None],
                            W1zT_sb[:, k, :],
                            start=(k == 0),
                            stop=(k == CT - 1),
                        )
                    nc.scalar.activation(
                        W6T_sb[0:1, :], ps_u[:1, 0:C], AF.Copy, scale=0.125
                    )

                    # layer 1: h = W6' @ Q (bias rows folded in), G-pair groups
                    for g in range(NK // G):
                        q_sb = qp.tile([8, G], BF16, tag="q")
                        nc.sync.dma_start(
                            out=q_sb, in_=d_Q[:, g * G : (g + 1) * G]
                        )
                        for m in range(CT):
                            ps = ps1.tile([P, G], F32, tag="ps1")
                            for c in range(G // 512):
                                nc.tensor.matmul(
                                    ps[:, c * 512 : (c + 1) * 512],
                                    W6T_sb[:, m * P : (m + 1) * P],
                                    q_sb[:, c * 512 : (c + 1) * 512],
                                    start=True,
                                    stop=True,
                                )
                            pair = prp.tile([P, G // K, K], BF16, tag="pair")
                            nc.scalar.activation(
                                pair.rearrange("p n k -> p (n k)"), ps,
                                AF.Prelu, bias=0.0, scale=1.0, alpha=alpha[:, :],
                            )
                            # tree-reduce sum over k (1/8 folded into weights)
                            t1 = prp.tile([P, G // K, 4], BF16, tag="t1")
                            nc.vector.tensor_add(
                                t1, pair[:, :, 0:4], pair[:, :, 4:8]
                            )
                            t2 = prp.tile([P, G // K, 2], BF16, tag="t2")
                            nc.vector.tensor_add(t2, t1[:, :, 0:2], t1[:, :, 2:4])
                            nc.vector.tensor_add(
                                X[:, m, g * (G // K) : (g + 1) * (G // K)],
                                t2[:, :, 0],
                                t2[:, :, 1],
                            )

            # ---------------- layers 2..4 ----------------
            psA_ctx = tc.tile_pool(name="psA", bufs=2, space="PSUM")
            psA = psA_ctx.__enter__()
            A_bf = ap_.tile([P, CT, N], BF16, tag="A")
            AT_bf = ap_.tile([P, NT, C], BF16, tag="AT")

            layer_params = [
                (W2T_sb, b2_sb, None),
                (W3T_sb, b3_sb, None),
                (W4T_sb, b4p_sb, s4_sb),
            ]
            NW = 512  # moving-operand width per matmul
            for li, (WT_sb, bias_sb, scale_sb) in enumerate(layer_params):
                last = li == len(layer_params) - 1
                # H = W @ X ; A = prelu(scale*H + bias)
                for m in range(CT):
                    ps = psA.tile([P, N], F32, tag="psA", name="psW")
                    for k in range(CT):
                        for nch in range(N // NW):
                            nc.tensor.matmul(
                                ps[:, nch * NW : (nch + 1) * NW],
                                WT_sb[:, k, m * P : (m + 1) * P],
                                X[:, k, nch * NW : (nch + 1) * NW],
                                start=(k == 0),
                                stop=(k == CT - 1),
                            )
                    nc.scalar.activation(
                        A_bf[:, m, :], ps,
                        AF.Prelu, bias=bias_sb[:, m, None],
                        scale=(scale_sb[:, m, None] if scale_sb is not None else 1.0),
                        alpha=alpha[:, :],
                    )
                # AT = A^T via DMA transpose (bf16)
                for m in range(CT):
                    nc.sync.dma_start(
                        out=AT_bf[:, :, m * P : (m + 1) * P],
                        in_=A_bf[:, m, :],
                        transpose=True,
                    )
                # X' = A @ S   (lhsT = AT blocks, rhs = S)
                for m in range(CT):
                    ps = psA.tile([P, N], F32, tag="psA", name="psS")
                    for k in range(NT):
                        for nch in range(N // NW):
                            nc.tensor.matmul(
                                ps[:, nch * NW : (nch + 1) * NW],
                                AT_bf[:, k, m * P : (m + 1) * P],
                                S_sb[:, k, nch * NW : (nch + 1) * NW],
                                start=(k == 0),
                                stop=(k == NT - 1),
                            )
                    nc.vector.tensor_copy(X[:, m, :], ps)
                    if last:
                        # fp32 feature output, cast during SWDGE DMA
                        nc.gpsimd.dma_start(
                            out=d_feat.rearrange("(t p) n -> p t n", p=P)[:, m, :],
                            in_=X[:, m, :],
                        )

            # ---------------- final mlp ----------------
            t5 = ap_.tile([64, N], F32, tag="t5")
            for nch in range(4):
                ps = psA.tile([P, N], F32, tag="psA", name="psM")
                for k in range(CT):
                    nc.tensor.matmul(
                        ps[:64, 0:512],
                        Wx1T_sb[:, k, :],
                        X[:, k, nch * 512 : (nch + 1) * 512],
                        start=(k == 0),
                        stop=(k == CT - 1),
                    )
                nc.scalar.activation(
                    t5[:, nch * 512 : (nch + 1) * 512], ps[:64, 0:512],
                    AF.Prelu, bias=bx1_sb, scale=1.0, alpha=alpha[:64, :],
                )
            sig = ap_.tile([3, N], F32, tag="sig")
            pcs_sb = ap_.tile([3, N], F32, tag="pcs")
            for nch in range(4):
                ps = psA.tile([P, N], F32, tag="psA", name="psM2")
                nc.tensor.matmul(
                    ps[:3, 0:512],
                    Wx2T_sb,
                    t5[:, nch * 512 : (nch + 1) * 512],
                    start=True,
                    stop=True,
                )
                nc.scalar.activation(
                    sig[:, nch * 512 : (nch + 1) * 512], ps[:3, 0:512],
                    AF.Sigmoid, bias=bx2_sb, scale=1.0,
                )
                nc.vector.tensor_scalar(
                    out=pcs_sb[:, nch * 512 : (nch + 1) * 512],
                    in0=sig[:, nch * 512 : (nch + 1) * 512],
                    scalar1=-0.5, scalar2=None, op0=ALU.add,
                )
                nc.sync.dma_start(
                    out=d_pcs[:, nch * 512 : (nch + 1) * 512],
                    in_=pcs_sb[:, nch * 512 : (nch + 1) * 512],
                )
            psA_ctx.__exit__(None, None, None)

    return nc


def _get_program():
    if "nc" not in _CACHED:
        _CACHED["nc"] = _build_program()
    return _CACHED["nc"]


def _prep_inputs(z, sphere, knn_idx, W1, b1, W2, b2, W3, b3, W4, b4,
                 bn_g, bn_b, bn_m, bn_v, Wx1, bx1, Wx2, bx2):
    bf = lambda a: np.ascontiguousarray(
        np.asarray(a, np.float32).astype(np.float16)
    )
    ptile = lambda a: np.ascontiguousarray(
        np.asarray(a, np.float32).reshape(-1, P, a.shape[-1]).transpose(1, 0, 2)
    )  # (T*P, M) -> (P, T, M)
    cvec = lambda a: np.asarray(a, np.float32).reshape(CT, P).T  # (C,) -> (P, CT)

    # averaging matrix S[m, n] = count(idx[n, :] == m) / 8
    idx = np.asarray(knn_idx, np.int64)
    S = np.zeros((N, N), np.float32)
    np.add.at(S, (idx, np.arange(N)[:, None]), 1.0 / K)

    xyz = np.asarray(sphere, np.float32).T                  # (3, N)
    Q = np.zeros((8, NK), np.float32)
    Q[0:2] = 1.0                                            # u and b1 rows
    Q[2:5] = np.repeat(xyz, K, axis=1)
    Q[5:8] = xyz[:, idx].reshape(3, NK)

    W1 = np.asarray(W1, np.float32)
    W1z = W1[:, 6:518]
    # rows: 0 = u/8 (device-filled), 1 = b1/8, 2:5 = (W1c-W1r)/8, 5:8 = W1r/8
    W6 = np.zeros((8, C), np.float32)
    W6[1] = np.asarray(b1, np.float32) / K
    W6[2:5] = (W1[:, 0:3] - W1[:, 3:6]).T / K
    W6[5:8] = W1[:, 3:6].T / K

    s4 = np.asarray(bn_g, np.float32) / np.sqrt(np.asarray(bn_v, np.float32) + BN_EPS)
    t4 = np.asarray(bn_b, np.float32) - np.asarray(bn_m, np.float32) * s4
    b4p = s4 * np.asarray(b4, np.float32) + t4

    z = np.asarray(z, np.float32)
    in_maps = []
    common = {
        "S": bf(ptile(S)),
        "Q": bf(Q),
        "W6T": bf(W6),
        "W1zT": bf(ptile(W1z.T)),
        "W2T": bf(ptile(np.asarray(W2, np.float32).T)),
        "W3T": bf(ptile(np.asarray(W3, np.float32).T)),
        "W4T": bf(ptile(np.asarray(W4, np.float32).T)),
        "Wx1T": bf(ptile(np.asarray(Wx1, np.float32).T)),
    }
    for i in range(z.shape[0]):
        bp = np.zeros((P, 29), np.float32)
        bp[:, 4:8] = cvec(b2)
        bp[:, 8:12] = cvec(b3)
        bp[:, 12:16] = cvec(s4)
        bp[:, 16:20] = cvec(b4p)
        bp[:, 20:24] = cvec(z[i])
        bp[:64, 24] = np.asarray(bx1, np.float32)
        bp[:3, 25] = np.asarray(bx2, np.float32)
        bp[:64, 26:29] = np.asarray(Wx2, np.float32).T
        in_maps.append(dict(common, bpack=np.ascontiguousarray(bp)))
    return in_maps


def kernel(z, sphere, knn_idx, W1, b1, W2, b2, W3, b3, W4, b4,
           bn_g, bn_b, bn_m, bn_v, Wx1, bx1, Wx2, bx2, point_num):
    B = np.asarray(z).shape[0]
    assert B == 8 and np.asarray(knn_idx).shape == (N, K)

    in_maps = _prep_inputs(z, sphere, knn_idx, W1, b1, W2, b2, W3, b3, W4, b4,
                           bn_g, bn_b, bn_m, bn_v, Wx1, bx1, Wx2, bx2)
    _CACHED["in_maps"] = in_maps

    nc = _get_program()
    res = run_bass_kernel_spmd(nc, in_maps, core_ids=list(range(B)))

    feature = np.stack([res.results[i]["feature"] for i in range(B)], 0)  # (B,C,N)
    pcs = np.stack([res.results[i]["pcs"] for i in range(B)], 0)          # (B,3,N)
    pcs = np.transpose(pcs, (0, 2, 1)).astype(np.float32)                 # (B,N,3)
    return (pcs, np.asarray(feature, np.float32))
